# revision 1
# baseline (speedup 1.0000x reference)
"""Causal multi-head self-attention with RoPE on 8 Trainium2 NeuronCores.

Sharding: batch (4) x query-half (2) -> 8 cores, no collectives.
Each core computes full K/V for its batch; query rows are split between the
two cores of a batch in a causally-balanced schedule (4 slots of 256 rows
with 16/12/8/4 key-blocks each), so both halves do equal attention work
under one shared SPMD program.  Causal masking is multiplicative mask input
data, so the same program serves both halves.

Everything is computed in transposed [feature, seq] layout so no on-device
transposes are needed:
  K^T/Q^T = W^T.T @ X^T          (per 128-row head pair)
  RoPE    = cos*x + sin*(P@x)    (P = constant pair-rotation matrix, one
                                  small extra matmul per tile)
  S^T     = Krot^T.T-slice @ Qrot^T  (keys on partitions -> softmax runs
                                      along the partition axis)
  exp     = ACT Exp(scale=1/8) -> bf16
  A^T,l   = [V|1].T-free matmul accumulated over key blocks in PSUM
  out     = A^T.T @ Wo^T         (natural [seq, feature] output layout)

Matmuls use fp32r (full PE rate at N>=256, ~FP22 precision) except the
attention-value matmul which is bf16 (exp output x V).
"""

import os
import sys
import math

if "/opt/trn_rl_repo" not in sys.path:
    sys.path.append("/opt/trn_rl_repo")

import numpy as np
import ml_dtypes

import concourse.bass as bass
import concourse.tile as tile
from concourse import bacc, mybir
from concourse.bass_utils import run_bass_kernel_spmd

B = 4
S = 2048
D = 1024
H = 16
DK = 64
THETA = 10000.0

NEP = H // 2          # head pairs (128-partition groups)
QT = 256              # query tile width (free dim of score matmuls)
KB = 128              # key block (partition dim of score output)
NSLOT = 4             # query slots per core
CNT = [16, 12, 8, 4]  # k-blocks per slot (uniform across cores)
TILES_J = [[7, 5, 3, 1], [6, 4, 2, 0]]  # 256-row q-tile indices per half
VW = DK + 1           # V columns per head incl. trailing ones column

F32R = mybir.dt.float32r
F32 = mybir.dt.float32
BF16 = mybir.dt.bfloat16

_cache = {}


def _build_program():
    if "nc" in _cache:
        return _cache["nc"]

    nc = bacc.Bacc("TRN2")

    xt_d = nc.dram_tensor("xt", [D, S], F32R, kind="ExternalInput")
    xq_d = nc.dram_tensor("xq", [D, NSLOT * QT], F32R, kind="ExternalInput")
    wkt_d = nc.dram_tensor("wkt", [D, D], F32R, kind="ExternalInput")
    wvt_d = nc.dram_tensor("wvt", [D, D], F32R, kind="ExternalInput")
    wqt_d = nc.dram_tensor("wqt", [D, D], F32R, kind="ExternalInput")
    wot_d = nc.dram_tensor("wot", [D, D], F32R, kind="ExternalInput")
    cosk_d = nc.dram_tensor("cosk", [128, S], F32R, kind="ExternalInput")
    sink_d = nc.dram_tensor("sink", [128, S], F32R, kind="ExternalInput")
    cosq_d = nc.dram_tensor("cosq", [128, NSLOT * QT], F32R, kind="ExternalInput")
    sinq_d = nc.dram_tensor("sinq", [128, NSLOT * QT], F32R, kind="ExternalInput")
    mask_d = nc.dram_tensor("mask", [128, NSLOT, 4, QT], BF16, kind="ExternalInput")
    permt_d = nc.dram_tensor("permt", [128, 128], F32R, kind="ExternalInput")
    ones_d = nc.dram_tensor("ones65", [VW, DK], F32R, kind="ExternalInput")
    y_d = nc.dram_tensor("y", [NSLOT * QT, D], F32, kind="ExternalOutput")

    def r(ap):
        return ap

    xt_t = xt_d.rearrange("(n p) s -> p n s", p=128)
    xq_t = xq_d.rearrange("(n p) s -> p n s", p=128)
    wkt_t = wkt_d.rearrange("(n p) e -> p n e", p=128)
    wqt_t = wqt_d.rearrange("(n p) e -> p n e", p=128)

    with tile.TileContext(nc) as tc:
        with (
            tc.tile_pool(name="kv", bufs=1) as kv,
            tc.tile_pool(name="const", bufs=1) as cpool,
        ):
            permt = cpool.tile([128, 128], F32R)
            nc.sync.dma_start(permt[:], permt_d[:])
            ones65 = cpool.tile([VW, DK], F32R)
            nc.sync.dma_start(ones65[:], ones_d[:])

            krot = [kv.tile([128, S], F32R, tag=f"krot{ep}", name=f"krot{ep}")
                    for ep in range(NEP)]
            vt = [kv.tile([128, H * VW], BF16, tag=f"vt{kb}", name=f"vt{kb}")
                  for kb in range(S // KB)]

            # ---------- Phase 1a: K^T projection + RoPE ----------
            with (
                tc.tile_pool(name="wk", bufs=1) as wkp,
                tc.tile_pool(name="ck", bufs=1) as ckp,
                tc.tile_pool(name="xs", bufs=3) as xsp,
                tc.tile_pool(name="t1a", bufs=2) as t1a,
                tc.tile_pool(name="ps1a", bufs=4, space="PSUM") as ps1a,
                tc.tile_pool(name="pp1a", bufs=2, space="PSUM") as pp1a,
            ):
                # K weights as per-head-pair column chunks: first matmul only
                # waits on one 0.5MB chunk instead of the full 4MB
                wk = [wkp.tile([128, 8, 128], F32R, tag=f"wk{ep}", name=f"wk{ep}")
                      for ep in range(NEP)]
                nc.sync.dma_start(wk[0][:], wkt_t[:, :, 0:128])
                xs_first = xsp.tile([128, 8, 512], F32R, tag="xs", name="xs_first")
                nc.gpsimd.dma_start(xs_first[:], xt_t[:, :, 0:512])
                cosk = ckp.tile([128, S], F32R)
                sink = ckp.tile([128, S], F32R)
                nc.sync.dma_start(cosk[:], cosk_d[:])
                nc.sync.dma_start(sink[:], sink_d[:])
                for ep in range(1, NEP):
                    nc.sync.dma_start(wk[ep][:], wkt_t[:, :, ep * 128:(ep + 1) * 128])
                def k_rope(kraw, ep, csl):
                    pp = pp1a.tile([128, 512], F32, tag="perm")
                    nc.tensor.matmul(pp[:], r(permt[:]), r(kraw[:]),
                                     start=True, stop=True)
                    t_c = t1a.tile([128, 512], F32R, tag="t_c")
                    nc.vector.tensor_mul(t_c[:], kraw[:], cosk[:, csl])
                    t_s = t1a.tile([128, 512], F32R, tag="t_s")
                    nc.vector.tensor_mul(t_s[:], pp[:], sink[:, csl])
                    nc.vector.tensor_add(krot[ep][:, csl], t_c[:], t_s[:])

                pend = []
                for st in range(S // 512):
                    if st == 0:
                        xs = xs_first
                    else:
                        xs = xsp.tile([128, 8, 512], F32R, tag="xs")
                        nc.gpsimd.dma_start(xs[:], xt_t[:, :, st * 512:(st + 1) * 512])
                    for ep in range(NEP):
                        pk = ps1a.tile([128, 512], F32, tag="proj")
                        for d in range(8):
                            nc.tensor.matmul(
                                pk[:], r(wk[ep][:, d, :]),
                                r(xs[:, d, :]), start=(d == 0), stop=(d == 7),
                            )
                        kraw = t1a.tile([128, 512], F32R, tag="kraw")
                        nc.any.tensor_copy(kraw[:], pk[:])
                        pend.append((kraw, ep, slice(st * 512, (st + 1) * 512)))
                        if len(pend) > 2:
                            k_rope(*pend.pop(0))
                for p_ in pend:
                    k_rope(*p_)

            # ---------- Phase 1b: V projection (+ones col) ----------
            with (
                tc.tile_pool(name="wv", bufs=1) as wvp,
                tc.tile_pool(name="xs2", bufs=3) as xsp2,
                tc.tile_pool(name="ps1b", bufs=4, space="PSUM") as ps1b,
            ):
                wvt = [wvp.tile([128, D], F32R, tag=f"wv{d}", name=f"wv{d}")
                       for d in range(8)]
                for d in range(8):
                    nc.sync.dma_start(wvt[d][:], wvt_d[d * 128:(d + 1) * 128, :])
                for kb in range(S // KB):
                    nc.vector.memset(
                        vt[kb].rearrange("p (h w) -> p h w", w=VW)[:, :, DK], 1.0
                    )
                for st in range(S // 512):
                    xs2 = xsp2.tile([128, 8, 512], F32R, tag="xs2")
                    nc.gpsimd.dma_start(xs2[:], xt_t[:, :, st * 512:(st + 1) * 512])
                    for half in range(4):
                        kb = 4 * st + half
                        off = half * KB
                        for et in range(2):
                            pv = ps1b.tile([128, 512], F32, tag="vproj")
                            for d in range(8):
                                nc.tensor.matmul(
                                    pv[:], r(xs2[:, d, off:off + KB]),
                                    r(wvt[d][:, et * 512:(et + 1) * 512]),
                                    start=(d == 0), stop=(d == 7),
                                )
                            dst = vt[kb].rearrange("p (h w) -> p h w", w=VW)
                            nc.any.tensor_copy(
                                dst[:, et * 8:(et + 1) * 8, 0:DK],
                                pv[:].rearrange("p (h w) -> p h w", w=DK),
                            )

            # ---------- Phase 1c..2 ----------
            with (
                tc.tile_pool(name="qp", bufs=1) as qp,
                tc.tile_pool(name="mk", bufs=1) as mkp,
            ):
                qrot = [qp.tile([128, NSLOT * QT], F32R, tag=f"qrot{ep}",
                                name=f"qrot{ep}") for ep in range(NEP)]
                masks = mkp.tile([128, NSLOT, 4, QT], BF16)
                nc.sync.dma_start(masks[:], mask_d[:])

                # ---------- Phase 1c: Q^T projection + RoPE ----------
                with (
                    tc.tile_pool(name="wq", bufs=1) as wqp,
                    tc.tile_pool(name="cq", bufs=1) as cqp,
                    tc.tile_pool(name="xs3", bufs=2) as xsp3,
                    tc.tile_pool(name="t1c", bufs=2) as t1c,
                    tc.tile_pool(name="ps1c", bufs=4, space="PSUM") as ps1c,
                    tc.tile_pool(name="pp1c", bufs=2, space="PSUM") as pp1c,
                ):
                    wq = [wqp.tile([128, 8, 128], F32R, tag=f"wq{ep}",
                                   name=f"wq{ep}") for ep in range(NEP)]
                    nc.sync.dma_start(wq[0][:], wqt_t[:, :, 0:128])
                    cosq = cqp.tile([128, NSLOT * QT], F32R)
                    sinq = cqp.tile([128, NSLOT * QT], F32R)
                    nc.sync.dma_start(cosq[:], cosq_d[:])
                    nc.sync.dma_start(sinq[:], sinq_d[:])
                    for ep in range(1, NEP):
                        nc.sync.dma_start(wq[ep][:],
                                          wqt_t[:, :, ep * 128:(ep + 1) * 128])

                    def q_rope(qraw, ep, csl):
                        pp = pp1c.tile([128, QT], F32, tag="qperm")
                        nc.tensor.matmul(pp[:], r(permt[:]), r(qraw[:]),
                                         start=True, stop=True)
                        t_c = t1c.tile([128, QT], F32R, tag="qt_c")
                        nc.vector.tensor_mul(t_c[:], qraw[:], cosq[:, csl])
                        t_s = t1c.tile([128, QT], F32R, tag="qt_s")
                        nc.vector.tensor_mul(t_s[:], pp[:], sinq[:, csl])
                        nc.vector.tensor_add(qrot[ep][:, csl], t_c[:], t_s[:])

                    pend = []
                    for qc in range(NSLOT):
                        xs3 = xsp3.tile([128, 8, QT], F32R, tag="xs3")
                        nc.gpsimd.dma_start(
                            xs3[:], xq_t[:, :, qc * QT:(qc + 1) * QT])
                        for ep in range(NEP):
                            pq = ps1c.tile([128, QT], F32, tag="qproj")
                            for d in range(8):
                                nc.tensor.matmul(
                                    pq[:], r(wq[ep][:, d, :]), r(xs3[:, d, :]),
                                    start=(d == 0), stop=(d == 7),
                                )
                            qraw = t1c.tile([128, QT], F32R, tag="qraw")
                            nc.any.tensor_copy(qraw[:], pq[:])
                            pend.append((qraw, ep, slice(qc * QT, (qc + 1) * QT)))
                            if len(pend) > 2:
                                q_rope(*pend.pop(0))
                    for p_ in pend:
                        q_rope(*p_)

                # ---------- Phase 2: attention + output projection ----------
                with (
                    tc.tile_pool(name="wo", bufs=1) as wop,
                    tc.tile_pool(name="at", bufs=1) as atp,
                    tc.tile_pool(name="ex", bufs=6) as exp_p,
                    tc.tile_pool(name="nrm", bufs=3) as nrmp,
                    tc.tile_pool(name="outs", bufs=3) as outs,
                    tc.tile_pool(name="ps_s", bufs=3, space="PSUM") as ps_s,
                    tc.tile_pool(name="ps_a", bufs=3, space="PSUM") as ps_a,
                    tc.tile_pool(name="ps_b", bufs=1, space="PSUM") as ps_b,
                    tc.tile_pool(name="ps_o", bufs=1, space="PSUM") as ps_o,
                ):
                    wot = [wop.tile([128, D], F32R, tag=f"wo{d}", name=f"wo{d}")
                           for d in range(8)]
                    for d in range(8):
                        nc.sync.dma_start(wot[d][:],
                                          wot_d[d * 128:(d + 1) * 128, :])

                    def normalize(accp, aT, ep):
                        lrow = nrmp.tile([VW, 2, QT], F32R, tag="lrow")
                        with nc.allow_low_precision(
                            reason="f32r tile holds full f32 bits"
                        ):
                            nc.vector.reciprocal(
                                lrow[DK:VW, :, :], accp[DK:VW, :, :]
                            )
                        pb_t = ps_b.tile([DK, 2, QT], F32, tag="bc")
                        nc.tensor.matmul(
                            pb_t[:], r(ones65[DK:VW, :]), r(lrow[DK:VW, :, :]),
                            start=True, stop=True,
                        )
                        rb = nrmp.tile([DK, 2, QT], F32, tag="rb")
                        nc.vector.tensor_copy(rb[:], pb_t[:])
                        nc.vector.tensor_mul(
                            aT[0:DK, :], accp[0:DK, 0, :], rb[:, 0, :]
                        )
                        tmp = nrmp.tile([DK, QT], F32R, tag="nt")
                        nc.vector.tensor_mul(
                            tmp[:], accp[0:DK, 1, :], rb[:, 1, :]
                        )
                        nc.gpsimd.dma_start(aT[DK:128, :], tmp[:])

                    pend_norm = None
                    for sl in range(NSLOT):
                        C = CNT[sl]
                        qsl = slice(sl * QT, (sl + 1) * QT)
                        aT = [atp.tile([128, QT], F32R, tag=f"aT{ep}",
                                       name=f"aT{ep}_{sl}")
                              for ep in range(NEP)]
                        for ep in range(NEP):
                            acc = [ps_a.tile([VW, QT], F32, tag="acc",
                                             name="acc") for _ in range(2)]
                            pend_ex = None

                            def flush_av(kb, exs):
                                for h in range(2):
                                    hh = 2 * ep + h
                                    nc.tensor.matmul(
                                        acc[h][:],
                                        vt[kb][:, hh * VW:(hh + 1) * VW],
                                        exs[h][:],
                                        start=(kb == 0), stop=(kb == C - 1),
                                    )

                            for kb in range(C):
                                exs = []
                                for h in range(2):
                                    pb = h * DK
                                    psc = ps_s.tile([128, QT], F32, tag="sc")
                                    nc.tensor.matmul(
                                        psc[:],
                                        r(krot[ep][pb:pb + DK, kb * KB:(kb + 1) * KB]),
                                        r(qrot[ep][pb:pb + DK, qsl]),
                                        start=True, stop=True,
                                        tile_position=(pb, 0),
                                    )
                                    e = exp_p.tile([128, QT], BF16, tag="ex")
                                    nc.scalar.activation(
                                        e[:], psc[:],
                                        mybir.ActivationFunctionType.Exp,
                                        scale=1.0 / math.sqrt(DK),
                                    )
                                    if kb >= C - 4:
                                        em = exp_p.tile([128, QT], BF16, tag="exm")
                                        nc.vector.tensor_mul(
                                            em[:], e[:], masks[:, sl, kb - (C - 4), :]
                                        )
                                        e = em
                                    exs.append(e)
                                if pend_ex is not None:
                                    flush_av(kb - 1, pend_ex)
                                pend_ex = exs
                            flush_av(C - 1, pend_ex)

                            for h in range(2):
                                lrow = nrmp.tile([VW, QT], F32R, tag="lrow")
                                with nc.allow_low_precision(
                                    reason="f32r holds full f32 bits"
                                ):
                                    nc.vector.reciprocal(
                                        lrow[DK:VW, :], acc[h][DK:VW, :]
                                    )
                                pb_t = ps_b.tile([DK, QT], F32, tag="bc")
                                nc.tensor.matmul(
                                    pb_t[:], r(ones65[DK:VW, :]), r(lrow[DK:VW, :]),
                                    start=True, stop=True,
                                )
                                rb = nrmp.tile([DK, QT], F32, tag="rb")
                                nc.vector.tensor_copy(rb[:], pb_t[:])
                                if h == 0:
                                    nc.vector.tensor_mul(
                                        aT[ep][0:DK, :], acc[h][0:DK, :], rb[:]
                                    )
                                else:
                                    tmp = nrmp.tile([DK, QT], F32R, tag="nt")
                                    nc.vector.tensor_mul(
                                        tmp[:], acc[h][0:DK, :], rb[:]
                                    )
                                    nc.sync.dma_start(aT[ep][DK:128, :], tmp[:])

                        for qs in range(2):
                            for et in range(2):
                                po = ps_o.tile([128, 512], F32, tag="out")
                                for d in range(8):
                                    nc.tensor.matmul(
                                        po[:],
                                        r(aT[d][:, qs * 128:(qs + 1) * 128]),
                                        r(wot[d][:, et * 512:(et + 1) * 512]),
                                        start=(d == 0), stop=(d == 7),
                                    )
                                ot = outs.tile([128, 512], F32, tag="ot")
                                nc.vector.tensor_copy(ot[:], po[:])
                                nc.sync.dma_start(
                                    y_d[sl * QT + qs * 128:
                                        sl * QT + (qs + 1) * 128,
                                        et * 512:(et + 1) * 512],
                                    ot[:],
                                )

    nc.compile()
    nc.finalize()
    _cache["nc"] = nc
    return nc


def _rope_tables(pos):
    """cos/sin tables in [128, n] head-pair layout (row e -> pair (e%64)//2)."""
    k = np.arange(DK // 2, dtype=np.float32)
    inv_freq = (THETA ** (-2.0 * k / DK)).astype(np.float32)
    ang = inv_freq[:, None] * pos.astype(np.float32)[None, :]  # [32, n]
    cos64 = np.repeat(np.cos(ang), 2, axis=0)
    sin64 = np.repeat(np.sin(ang), 2, axis=0)
    return (np.ascontiguousarray(np.concatenate([cos64, cos64], axis=0)),
            np.ascontiguousarray(np.concatenate([sin64, sin64], axis=0)))


def _masks(j):
    """[128, NSLOT, 4, QT] bf16 multiplicative causal masks for half j."""
    p = np.arange(KB)[:, None]
    f = np.arange(QT)[None, :]
    triA = (f >= p).astype(np.float32)
    triB = (f >= p + KB).astype(np.float32)
    ones = np.ones((KB, QT), np.float32)
    zeros = np.zeros((KB, QT), np.float32)
    per_slot = [ones, ones, triA, triB] if j == 0 else [triA, triB, zeros, zeros]
    m = np.stack([np.stack(per_slot, axis=0)] * NSLOT, axis=0)  # [slot, 4, p, f]
    return np.ascontiguousarray(
        m.transpose(2, 0, 1, 3)).astype(ml_dtypes.bfloat16)


def _host_inputs(in_features, token_positions, Wq, Wk, Wv, Wo):
    X = np.asarray(in_features, dtype=np.float32)
    pos = np.asarray(token_positions)
    wqt = np.ascontiguousarray(np.asarray(Wq, np.float32).T)
    wkt = np.ascontiguousarray(np.asarray(Wk, np.float32).T)
    wvt = np.ascontiguousarray(np.asarray(Wv, np.float32).T)
    wot = np.ascontiguousarray(np.asarray(Wo, np.float32).T)
    cosk, sink = _rope_tables(pos)

    permt = np.zeros((128, 128), np.float32)
    for i in range(64):
        permt[2 * i + 1, 2 * i] = -1.0
        permt[2 * i, 2 * i + 1] = 1.0

    in_maps = []
    for core in range(8):
        b, j = core // 2, core % 2
        rows = np.concatenate(
            [np.arange(t * QT, (t + 1) * QT) for t in TILES_J[j]])
        cosq, sinq = _rope_tables(pos[rows])
        in_maps.append({
            "xt": np.ascontiguousarray(X[b].T),
            "xq": np.ascontiguousarray(X[b][rows].T),
            "wkt": wkt, "wvt": wvt, "wqt": wqt, "wot": wot,
            "cosk": cosk, "sink": sink, "cosq": cosq, "sinq": sinq,
            "mask": _masks(j), "permt": permt,
            "ones65": np.ones((VW, DK), np.float32),
        })
    return in_maps


def kernel(in_features, token_positions, Wq, Wk, Wv, Wo):
    nc = _build_program()
    in_maps = _host_inputs(in_features, token_positions, Wq, Wk, Wv, Wo)

    trace = bool(int(os.environ.get("KERNEL_TRACE", "0")))
    res = run_bass_kernel_spmd(nc, in_maps, core_ids=list(range(8)), trace=trace)
    kernel.last_result = res

    out = np.empty((B, S, D), np.float32)
    for core in range(8):
        b, j = core // 2, core % 2
        y = res.results[core]["y"]
        for s_i, t in enumerate(TILES_J[j]):
            out[b, t * QT:(t + 1) * QT, :] = y[s_i * QT:(s_i + 1) * QT, :]
    return out



# revision 17
# speedup vs baseline: 1.3603x; 1.3603x over previous
"""Causal multi-head self-attention with RoPE on 8 Trainium2 NeuronCores.

Sharding: batch (4) x head-half (2) -> 8 cores, no collectives.
Each core owns one batch element and 8 of the 16 heads (4 head-pairs of
128 partitions).  It computes K/V/Q projections for its heads only (no
redundant K/V work), full causal attention over all 8 query tiles, and a
row-sharded output projection producing a PARTIAL [S, D] output; the two
partials of a batch are summed on the host.

Pipeline: the program is emitted per 256-row sequence chunk c (== query
tile t).  Projection work for chunk t+1 is split into small sub-units and
injected into the attention inner loop of tile t, so the in-order PE
queue always has independent work between dependent attention steps.
The activation engine runs ONLY exp (both heads merged into one 512-wide
instruction reading a two-bank PSUM pair); rope/masking multiply on
Pool, PSUM-reading copies and normalization on DVE.

Layouts (transposed, feature-on-partition; no on-device transposes):
  K^T/Q^T = W^T.T @ X^T         bf16 inputs, fp32 PSUM accumulate
  RoPE    = cos*x + sin*(P@x)   (P = pair-rotation matrix) -> bf16
  S^T     = Krot^T-slice.T @ Qrot^T   (keys on partitions)
  exp     = ACT Exp(scale=1/8) over [128, 2heads, 256] -> bf16
  A^T,l   = [V|1].T @ exp       accumulated over key blocks in PSUM
  out     = A^T.T @ Wo^T        partial, host-reduced across head halves
"""

import os
import sys
import math

if "/opt/trn_rl_repo" not in sys.path:
    sys.path.append("/opt/trn_rl_repo")

import numpy as np
import ml_dtypes

import concourse.bass as bass
import concourse.tile as tile
from concourse import bacc, mybir
from concourse.bass_utils import run_bass_kernel_spmd

B = 4
S = 2048
D = 1024
H = 16
DK = 64
THETA = 10000.0

NEP = 4               # head pairs per core (8 heads)
QT = 256              # query tile width == chunk width
KB = 128              # key block
NT = S // QT          # 8 query tiles
VW = DK + 1           # V columns per head incl. trailing ones column

F32R = mybir.dt.float32r
F32 = mybir.dt.float32
F16 = mybir.dt.float16
BF16 = mybir.dt.bfloat16

_cache = {}
NO_INJECT = bool(int(os.environ.get("KERNEL_NO_INJECT", "0")))


def _build_program():
    if "nc" in _cache:
        return _cache["nc"]

    nc = bacc.Bacc("TRN2")

    xt_d = nc.dram_tensor("xt", [D, S], BF16, kind="ExternalInput")
    wkt_d = nc.dram_tensor("wkt", [D, NEP * 128], BF16, kind="ExternalInput")
    wvt_d = nc.dram_tensor("wvt", [D, NEP * 128], BF16, kind="ExternalInput")
    wqt_d = nc.dram_tensor("wqt", [D, NEP * 128], BF16, kind="ExternalInput")
    wot_d = nc.dram_tensor("wot", [NEP * 128, D], BF16, kind="ExternalInput")
    cost_d = nc.dram_tensor("cost", [128, S], F16, kind="ExternalInput")
    sint_d = nc.dram_tensor("sint", [128, S], F16, kind="ExternalInput")
    mask_d = nc.dram_tensor("mask", [128, 2, 2, QT], BF16, kind="ExternalInput")
    permt_d = nc.dram_tensor("permt", [128, 128], F32R, kind="ExternalInput")
    ones_d = nc.dram_tensor("ones65", [VW, DK], F32R, kind="ExternalInput")
    y_d = nc.dram_tensor("y", [S, D], BF16, kind="ExternalOutput")

    xt_t = xt_d.rearrange("(n p) s -> p n s", p=128)      # [128, 8, S]
    wkt_t = wkt_d.rearrange("(n p) e -> p n e", p=128)    # [128, 8, 512]
    wqt_t = wqt_d.rearrange("(n p) e -> p n e", p=128)
    wvt_t = wvt_d.rearrange("(n p) e -> p n e", p=128)
    wot_t = wot_d.rearrange("(n p) e -> p n e", p=128)    # [128, 4, 1024]

    with tile.TileContext(nc) as tc:
        with (
            tc.tile_pool(name="const", bufs=1) as cpool,
            tc.tile_pool(name="wpool", bufs=1) as wpool,
            tc.tile_pool(name="kv", bufs=1) as kv,
            tc.tile_pool(name="xsp", bufs=3) as xsp,
            tc.tile_pool(name="t1", bufs=2) as t1,
            tc.tile_pool(name="atp", bufs=2) as atp,
            tc.tile_pool(name="ps_sc", bufs=2, space="PSUM") as ps_sc,
            tc.tile_pool(name="ps_pj", bufs=2, space="PSUM") as ps_pj,
            tc.tile_pool(name="ps_ac", bufs=1, space="PSUM") as ps_ac,
        ):
            # ---------------- persistent tiles ----------------
            permt = cpool.tile([128, 128], F32R)
            ones65 = cpool.tile([VW, DK], F32R)
            cost = cpool.tile([128, S], F16)
            sint = cpool.tile([128, S], F16)
            masks = cpool.tile([128, 2, 2, QT], BF16)

            wk = [wpool.tile([128, 8, 128], BF16, tag=f"wk{e}", name=f"wk{e}")
                  for e in range(NEP)]
            wq = [wpool.tile([128, 8, 128], BF16, tag=f"wq{e}", name=f"wq{e}")
                  for e in range(NEP)]
            wv = [wpool.tile([128, NEP * 128], BF16, tag=f"wv{d}", name=f"wv{d}")
                  for d in range(8)]
            wo = [wpool.tile([128, D], BF16, tag=f"wo{d}", name=f"wo{d}")
                  for d in range(NEP)]

            krot = [kv.tile([128, S], BF16, tag=f"krot{e}", name=f"krot{e}")
                    for e in range(NEP)]
            qrot = [kv.tile([128, S], BF16, tag=f"qrot{e}", name=f"qrot{e}")
                    for e in range(NEP)]
            vt = [kv.tile([128, 8, VW], BF16, tag=f"vt{kb}", name=f"vt{kb}")
                  for kb in range(S // KB)]

            # ---------------- prologue DMAs ----------------
            xs_tiles = {}

            def xs_dma(c):
                xs = xsp.tile([128, 8, QT], BF16, tag="xs", name=f"xs{c}")
                nc.sync.dma_start(xs[:], xt_t[:, :, c * QT:(c + 1) * QT])
                xs_tiles[c] = xs

            nc.sync.dma_start(wk[0][:], wkt_t[:, :, 0:128])
            xs_dma(0)
            nc.sync.dma_start(cost[:], cost_d[:])
            nc.sync.dma_start(sint[:], sint_d[:])
            nc.sync.dma_start(permt[:], permt_d[:])
            for e in range(1, NEP):
                nc.sync.dma_start(wk[e][:], wkt_t[:, :, e * 128:(e + 1) * 128])
            for d in range(8):
                nc.sync.dma_start(wv[d][:], wvt_t[:, d, :])
            for e in range(NEP):
                nc.sync.dma_start(wq[e][:], wqt_t[:, :, e * 128:(e + 1) * 128])
            nc.sync.dma_start(masks[:], mask_d[:])
            nc.sync.dma_start(ones65[:], ones_d[:])
            xs_dma(1)
            xs_dma(2)
            for d in range(NEP):
                nc.sync.dma_start(wo[d][:], wot_t[:, d, :])
            for kb in range(S // KB):
                nc.vector.memset(vt[kb][:, :, DK], 1.0)

            # ---------------- unit emitters ----------------
            # K/Q projection is split into two sub-units so the PSUM->SBUF
            # copy latency is hidden behind other injected PE work.
            raws = {}

            def proj_a(c, ep, wtile, raw_tag):
                pk = ps_pj.tile([128, 512], F32, tag="pj")
                for d in range(8):
                    nc.tensor.matmul(pk[:, 0:QT], wtile[:, d, :],
                                     xs_tiles[c][:, d, :],
                                     start=(d == 0), stop=(d == 7))
                raw = t1.tile([128, QT], F32R, tag=raw_tag)
                nc.vector.tensor_copy(raw[:], pk[:, 0:QT])
                raws[(c, ep, raw_tag)] = raw

            def proj_b(c, ep, out, raw_tag):
                csl = slice(c * QT, (c + 1) * QT)
                raw = raws.pop((c, ep, raw_tag))
                pp = ps_pj.tile([128, 512], F32, tag="pj")
                nc.tensor.matmul(pp[:, 0:QT], permt[:], raw[:],
                                 start=True, stop=True)
                t_c = t1.tile([128, QT], BF16, tag="tc")
                nc.gpsimd.tensor_mul(t_c[:], raw[:], cost[:, csl])
                t_s = t1.tile([128, QT], BF16, tag="ts")
                nc.vector.tensor_mul(t_s[:], pp[:, 0:QT], sint[:, csl])
                nc.gpsimd.tensor_add(out[:, csl], t_c[:], t_s[:])

            def emit_V(c, half):
                kb = 2 * c + half
                off = half * KB
                pv = ps_pj.tile([128, 512], F32, tag="pj")
                for d in range(8):
                    nc.tensor.matmul(pv[:], xs_tiles[c][:, d, off:off + KB],
                                     wv[d][:], start=(d == 0), stop=(d == 7))
                nc.vector.tensor_copy(
                    vt[kb][:, :, 0:DK],
                    pv[:].rearrange("p (h w) -> p h w", w=DK))

            def units_for_chunk(c):
                us = []
                for ep in range(NEP):
                    us.append(lambda c=c, ep=ep: proj_a(c, ep, wk[ep], "kraw"))
                    us.append(lambda c=c, ep=ep: proj_b(c, ep, krot[ep], "kraw"))
                us.insert(2, lambda c=c: emit_V(c, 0))
                us.insert(6, lambda c=c: emit_V(c, 1))
                for ep in range(NEP):
                    us.append(lambda c=c, ep=ep: proj_a(c, ep, wq[ep], "qraw"))
                    us.append(lambda c=c, ep=ep: proj_b(c, ep, qrot[ep], "qraw"))
                return us

            aT_tiles = {}

            def emit_po(t, qs, et):
                po = ps_pj.tile([128, 512], F32, tag="pj")
                for d in range(NEP):
                    nc.tensor.matmul(
                        po[:], aT_tiles[(t, d)][:, qs * 128:(qs + 1) * 128],
                        wo[d][:, et * 512:(et + 1) * 512],
                        start=(d == 0), stop=(d == NEP - 1))
                ot = t1.tile([128, 512], BF16, tag="ot")
                nc.vector.tensor_copy(ot[:], po[:])
                nc.sync.dma_start(
                    y_d[t * QT + qs * 128: t * QT + (qs + 1) * 128,
                        et * 512:(et + 1) * 512], ot[:])

            def pop(queue, n=1):
                for _ in range(n):
                    if queue:
                        queue.pop(0)()

            def attn(t, ep, queue, qstate):
                C = 2 * (t + 1)
                qsl = slice(t * QT, (t + 1) * QT)
                acc = [ps_ac.tile([VW, QT], F32, tag=f"ac{h}",
                                  name=f"acc{h}") for h in range(2)]
                pend = None
                for kb in range(C):
                    e = t1.tile([128, 2, QT], BF16, tag="e", bufs=3)
                    psc = ps_sc.tile([128, 2, 512], F32, tag="sc", bufs=2)
                    for h in range(2):
                        pb = h * DK
                        nc.tensor.matmul(
                            psc[:, h, 0:QT],
                            krot[ep][pb:pb + DK, kb * KB:(kb + 1) * KB],
                            qrot[ep][pb:pb + DK, qsl],
                            start=True, stop=True,
                            tile_position=(pb, 0))
                    nc.scalar.activation(e[:], psc[:, :, 0:QT],
                                         mybir.ActivationFunctionType.Exp,
                                         scale=1.0 / math.sqrt(DK))
                    if kb >= C - 2:
                        em = t1.tile([128, 2, QT], BF16, tag="em")
                        nc.gpsimd.tensor_mul(em[:], e[:],
                                             masks[:, kb - (C - 2), :, :])
                        e = em
                    if pend is not None:
                        pkb, pe = pend
                        for h in range(2):
                            nc.tensor.matmul(
                                acc[h][:], vt[pkb][:, 2 * ep + h, :],
                                pe[:, h, :],
                                start=(pkb == 0), stop=(pkb == C - 1))
                    pend = (kb, e)
                    if not NO_INJECT:
                        qstate[0] += qstate[1]
                        while queue and qstate[0] >= 1.0:
                            qstate[0] -= 1.0
                            queue.pop(0)()
                if not NO_INJECT:
                    pop(queue)
                pkb, pe = pend
                for h in range(2):
                    nc.tensor.matmul(
                        acc[h][:], vt[pkb][:, 2 * ep + h, :], pe[:, h, :],
                        start=(pkb == 0), stop=(pkb == C - 1))

                # normalization: aT = acc / rowsum  (rowsum in acc row DK)
                lrow = t1.tile([VW, 2, QT], F32R, tag="lrow")
                with nc.allow_low_precision(
                    reason="f32r tile holds full f32 bits"
                ):
                    for h in range(2):
                        nc.vector.reciprocal(lrow[DK:VW, h, :],
                                             acc[h][DK:VW, :])
                if not NO_INJECT:
                    pop(queue)
                pbt = ps_pj.tile([128, 2, QT], F32, tag="pj")
                nc.tensor.matmul(pbt[0:DK, :, :], ones65[DK:VW, :],
                                 lrow[DK:VW, :, :], start=True, stop=True)
                rb = t1.tile([DK, 2, QT], F32, tag="rb")
                nc.vector.tensor_copy(rb[:], pbt[0:DK, :, :])
                aT = atp.tile([128, QT], BF16, tag=f"aT{ep}", name=f"aT{ep}_{t}")
                aT_tiles[(t, ep)] = aT
                nc.vector.tensor_mul(aT[0:DK, :], acc[0][0:DK, :], rb[:, 0, :])
                tmp = t1.tile([DK, QT], BF16, tag="tmp")
                nc.vector.tensor_mul(tmp[:], acc[1][0:DK, :], rb[:, 1, :])
                nc.sync.dma_start(aT[DK:128, :], tmp[:])

            # ---------------- main pipeline ----------------
            for fn in units_for_chunk(0):
                fn()

            for t in range(NT):
                queue = []
                if t + 3 <= NT - 1:
                    xs_dma(t + 3)
                if t + 1 < NT:
                    queue.extend(units_for_chunk(t + 1))
                if t >= 1:
                    for qs in range(2):
                        for et in range(2):
                            queue.append(
                                lambda t=t - 1, qs=qs, et=et: emit_po(t, qs, et))
                niter = 4 * 2 * (t + 1)
                qstate = [0.0, len(queue) / max(1, niter)]
                if NO_INJECT:
                    for fn in queue:
                        fn()
                    queue = []
                for ep in range(NEP):
                    attn(t, ep, queue, qstate)
                for fn in queue:
                    fn()
            for qs in range(2):
                for et in range(2):
                    emit_po(NT - 1, qs, et)

    nc.compile()
    nc.finalize()
    _cache["nc"] = nc
    return nc


def _rope_tables(pos):
    """cos/sin tables in [128, n] head-pair layout."""
    k = np.arange(DK // 2, dtype=np.float32)
    inv_freq = (THETA ** (-2.0 * k / DK)).astype(np.float32)
    ang = inv_freq[:, None] * pos.astype(np.float32)[None, :]  # [32, n]
    cos64 = np.repeat(np.cos(ang), 2, axis=0)
    sin64 = np.repeat(np.sin(ang), 2, axis=0)
    return (np.ascontiguousarray(
                np.concatenate([cos64, cos64], axis=0)).astype(np.float16),
            np.ascontiguousarray(
                np.concatenate([sin64, sin64], axis=0)).astype(np.float16))


def _masks():
    """[128, 2, 2, QT] bf16 multiplicative diagonal-block causal masks."""
    p = np.arange(KB)[:, None]
    f = np.arange(QT)[None, :]
    triA = (f >= p).astype(np.float32)
    triB = (f >= p + KB).astype(np.float32)
    m = np.stack([np.stack([triA, triA], 0), np.stack([triB, triB], 0)], 0)
    return np.ascontiguousarray(
        m.transpose(2, 0, 1, 3)).astype(ml_dtypes.bfloat16)


def _host_inputs(in_features, token_positions, Wq, Wk, Wv, Wo):
    X = np.asarray(in_features, dtype=np.float32)
    pos = np.asarray(token_positions)
    bf = ml_dtypes.bfloat16
    wqT = np.ascontiguousarray(np.asarray(Wq, np.float32).T).astype(bf)
    wkT = np.ascontiguousarray(np.asarray(Wk, np.float32).T).astype(bf)
    wvT = np.ascontiguousarray(np.asarray(Wv, np.float32).T).astype(bf)
    woT = np.ascontiguousarray(np.asarray(Wo, np.float32).T).astype(bf)
    cost, sint = _rope_tables(pos)

    permt = np.zeros((128, 128), np.float32)
    for i in range(64):
        permt[2 * i + 1, 2 * i] = -1.0
        permt[2 * i, 2 * i + 1] = 1.0

    mask = _masks()
    in_maps = []
    for core in range(8):
        b, j = core // 2, core % 2
        cs = slice(512 * j, 512 * (j + 1))
        in_maps.append({
            "xt": np.ascontiguousarray(X[b].T).astype(bf),
            "wkt": np.ascontiguousarray(wkT[:, cs]),
            "wvt": np.ascontiguousarray(wvT[:, cs]),
            "wqt": np.ascontiguousarray(wqT[:, cs]),
            "wot": np.ascontiguousarray(woT[cs, :]),
            "cost": cost, "sint": sint,
            "mask": mask, "permt": permt,
            "ones65": np.ones((VW, DK), np.float32),
        })
    return in_maps


def kernel(in_features, token_positions, Wq, Wk, Wv, Wo):
    nc = _build_program()
    in_maps = _host_inputs(in_features, token_positions, Wq, Wk, Wv, Wo)

    trace = bool(int(os.environ.get("KERNEL_TRACE", "0")))
    res = run_bass_kernel_spmd(nc, in_maps, core_ids=list(range(8)), trace=trace)
    kernel.last_result = res

    out = np.empty((B, S, D), np.float32)
    for b in range(B):
        out[b] = (res.results[2 * b]["y"].astype(np.float32)
                  + res.results[2 * b + 1]["y"].astype(np.float32))
    return out


# revision 18
# speedup vs baseline: 1.6062x; 1.1808x over previous
"""Causal multi-head self-attention with RoPE on 8 Trainium2 NeuronCores.

Sharding: batch (4) x head-half (2) -> 8 cores, no collectives.
Each core owns one batch element and 8 of the 16 heads (4 head-pairs of
128 partitions).  It computes K/V/Q projections for its heads only (no
redundant K/V work), full causal attention over all 8 query tiles, and a
row-sharded output projection producing a PARTIAL [S, D] output; the two
partials of a batch are summed on the host.

Pipeline: the program is emitted per 256-row sequence chunk c (== query
tile t).  Projection work for chunk t+1 is split into small sub-units and
injected into the attention inner loop of tile t, so the in-order PE
queue always has independent work between dependent attention steps.
The activation engine runs ONLY exp (both heads merged into one 512-wide
instruction reading a two-bank PSUM pair); rope/masking multiply on
Pool, PSUM-reading copies and normalization on DVE.

Layouts (transposed, feature-on-partition; no on-device transposes):
  K^T/Q^T = W^T.T @ X^T         bf16 inputs, fp32 PSUM accumulate
  RoPE    = cos*x + sin*(P@x)   (P = pair-rotation matrix) -> bf16
  S^T     = Krot^T-slice.T @ Qrot^T   (keys on partitions)
  exp     = ACT Exp(scale=1/8) over [128, 2heads, 256] -> bf16
  A^T,l   = [V|1].T @ exp       accumulated over key blocks in PSUM
  out     = A^T.T @ Wo^T        partial, host-reduced across head halves
"""

import os
import sys
import math

if "/opt/trn_rl_repo" not in sys.path:
    sys.path.append("/opt/trn_rl_repo")

import numpy as np
import ml_dtypes

import concourse.bass as bass
import concourse.tile as tile
from concourse import bacc, mybir
from concourse.bass_utils import run_bass_kernel_spmd

B = 4
S = 2048
D = 1024
H = 16
DK = 64
THETA = 10000.0

NEP = 4               # head pairs per core (8 heads)
QT = 256              # query tile width == chunk width
KB = 128              # key block
NT = S // QT          # 8 query tiles
VW = DK + 1           # V columns per head incl. trailing ones column

F32R = mybir.dt.float32r
F32 = mybir.dt.float32
F16 = mybir.dt.float16
BF16 = mybir.dt.bfloat16

_cache = {}
NO_INJECT = bool(int(os.environ.get("KERNEL_NO_INJECT", "0")))


def _build_program():
    if "nc" in _cache:
        return _cache["nc"]

    nc = bacc.Bacc("TRN2")

    xt_d = nc.dram_tensor("xt", [D, S], BF16, kind="ExternalInput")
    wkt_d = nc.dram_tensor("wkt", [D, NEP * 128], BF16, kind="ExternalInput")
    wvt_d = nc.dram_tensor("wvt", [D, NEP * 128], BF16, kind="ExternalInput")
    wqt_d = nc.dram_tensor("wqt", [D, NEP * 128], BF16, kind="ExternalInput")
    wot_d = nc.dram_tensor("wot", [NEP * 128, D], BF16, kind="ExternalInput")
    cost_d = nc.dram_tensor("cost", [128, S], F16, kind="ExternalInput")
    sint_d = nc.dram_tensor("sint", [128, S], F16, kind="ExternalInput")
    mask_d = nc.dram_tensor("mask", [128, 2, 2, QT], BF16, kind="ExternalInput")
    permt_d = nc.dram_tensor("permt", [128, 128], F32R, kind="ExternalInput")
    ones_d = nc.dram_tensor("ones65", [VW, DK], F32R, kind="ExternalInput")
    y_d = nc.dram_tensor("y", [S, D], BF16, kind="ExternalOutput")

    xt_t = xt_d.rearrange("(n p) s -> p n s", p=128)      # [128, 8, S]
    wkt_t = wkt_d.rearrange("(n p) e -> p n e", p=128)    # [128, 8, 512]
    wqt_t = wqt_d.rearrange("(n p) e -> p n e", p=128)
    wvt_t = wvt_d.rearrange("(n p) e -> p n e", p=128)
    wot_t = wot_d.rearrange("(n p) e -> p n e", p=128)    # [128, 4, 1024]

    with tile.TileContext(nc) as tc:
        with (
            tc.tile_pool(name="const", bufs=1) as cpool,
            tc.tile_pool(name="wpool", bufs=1) as wpool,
            tc.tile_pool(name="kv", bufs=1) as kv,
            tc.tile_pool(name="xsp", bufs=3) as xsp,
            tc.tile_pool(name="t1", bufs=2) as t1,
            tc.tile_pool(name="atp", bufs=2) as atp,
            tc.tile_pool(name="ps_sc", bufs=2, space="PSUM") as ps_sc,
            tc.tile_pool(name="ps_pj", bufs=2, space="PSUM") as ps_pj,
            tc.tile_pool(name="ps_ac", bufs=1, space="PSUM") as ps_ac,
        ):
            # ---------------- persistent tiles ----------------
            permt = cpool.tile([128, 128], F32R)
            ones65 = cpool.tile([VW, DK], F32R)
            cost = cpool.tile([128, S], F16)
            sint = cpool.tile([128, S], F16)
            masks = cpool.tile([128, 2, 2, QT], BF16)

            wk = [wpool.tile([128, 8, 128], BF16, tag=f"wk{e}", name=f"wk{e}")
                  for e in range(NEP)]
            wq = [wpool.tile([128, 8, 128], BF16, tag=f"wq{e}", name=f"wq{e}")
                  for e in range(NEP)]
            wv = [wpool.tile([128, NEP * 128], BF16, tag=f"wv{d}", name=f"wv{d}")
                  for d in range(8)]
            wo = [wpool.tile([128, D], BF16, tag=f"wo{d}", name=f"wo{d}")
                  for d in range(NEP)]

            krot = [kv.tile([128, S], BF16, tag=f"krot{e}", name=f"krot{e}")
                    for e in range(NEP)]
            qrot = [kv.tile([128, S], BF16, tag=f"qrot{e}", name=f"qrot{e}")
                    for e in range(NEP)]
            vt = [kv.tile([128, 8, VW], BF16, tag=f"vt{kb}", name=f"vt{kb}")
                  for kb in range(S // KB)]

            # ---------------- prologue DMAs ----------------
            xs_tiles = {}

            def xs_dma(c):
                xs = xsp.tile([128, 8, QT], BF16, tag="xs", name=f"xs{c}")
                nc.sync.dma_start(xs[:], xt_t[:, :, c * QT:(c + 1) * QT])
                xs_tiles[c] = xs

            nc.sync.dma_start(wk[0][:], wkt_t[:, :, 0:128])
            xs_dma(0)
            nc.sync.dma_start(cost[:], cost_d[:])
            nc.sync.dma_start(sint[:], sint_d[:])
            nc.sync.dma_start(permt[:], permt_d[:])
            for e in range(1, NEP):
                nc.sync.dma_start(wk[e][:], wkt_t[:, :, e * 128:(e + 1) * 128])
            for d in range(8):
                nc.sync.dma_start(wv[d][:], wvt_t[:, d, :])
            for e in range(NEP):
                nc.sync.dma_start(wq[e][:], wqt_t[:, :, e * 128:(e + 1) * 128])
            nc.sync.dma_start(masks[:], mask_d[:])
            nc.sync.dma_start(ones65[:], ones_d[:])
            xs_dma(1)
            xs_dma(2)
            for d in range(NEP):
                nc.sync.dma_start(wo[d][:], wot_t[:, d, :])
            for kb in range(S // KB):
                nc.vector.memset(vt[kb][:, :, DK], 1.0)

            # ---------------- unit emitters ----------------
            # K/Q projection is split into two sub-units so the PSUM->SBUF
            # copy latency is hidden behind other injected PE work.
            raws = {}

            def proj_a(c, ep, wtile, raw_tag):
                pk = ps_pj.tile([128, 512], F32, tag="pj")
                for d in range(8):
                    nc.tensor.matmul(pk[:, 0:QT], wtile[:, d, :],
                                     xs_tiles[c][:, d, :],
                                     start=(d == 0), stop=(d == 7))
                raw = t1.tile([128, QT], F32R, tag=raw_tag)
                nc.vector.tensor_copy(raw[:], pk[:, 0:QT])
                raws[(c, ep, raw_tag)] = raw

            def proj_b(c, ep, out, raw_tag):
                csl = slice(c * QT, (c + 1) * QT)
                raw = raws.pop((c, ep, raw_tag))
                pp = ps_pj.tile([128, 512], F32, tag="pj")
                nc.tensor.matmul(pp[:, 0:QT], permt[:], raw[:],
                                 start=True, stop=True)
                t_c = t1.tile([128, QT], BF16, tag="tc")
                nc.gpsimd.tensor_mul(t_c[:], raw[:], cost[:, csl])
                t_s = t1.tile([128, QT], BF16, tag="ts")
                nc.vector.tensor_mul(t_s[:], pp[:, 0:QT], sint[:, csl])
                nc.gpsimd.tensor_add(out[:, csl], t_c[:], t_s[:])

            def emit_V(c, half):
                kb = 2 * c + half
                off = half * KB
                pv = ps_pj.tile([128, 512], F32, tag="pj")
                for d in range(8):
                    nc.tensor.matmul(pv[:], xs_tiles[c][:, d, off:off + KB],
                                     wv[d][:], start=(d == 0), stop=(d == 7))
                nc.vector.tensor_copy(
                    vt[kb][:, :, 0:DK],
                    pv[:].rearrange("p (h w) -> p h w", w=DK))

            def units_for_chunk(c):
                us = []
                for ep in range(NEP):
                    us.append(lambda c=c, ep=ep: proj_a(c, ep, wk[ep], "kraw"))
                    us.append(lambda c=c, ep=ep: proj_b(c, ep, krot[ep], "kraw"))
                us.insert(2, lambda c=c: emit_V(c, 0))
                us.insert(6, lambda c=c: emit_V(c, 1))
                for ep in range(NEP):
                    us.append(lambda c=c, ep=ep: proj_a(c, ep, wq[ep], "qraw"))
                    us.append(lambda c=c, ep=ep: proj_b(c, ep, qrot[ep], "qraw"))
                return us

            aT_tiles = {}

            def emit_po(t, qs, et):
                po = ps_pj.tile([128, 512], F32, tag="pj")
                for d in range(NEP):
                    nc.tensor.matmul(
                        po[:], aT_tiles[(t, d)][:, qs * 128:(qs + 1) * 128],
                        wo[d][:, et * 512:(et + 1) * 512],
                        start=(d == 0), stop=(d == NEP - 1))
                ot = t1.tile([128, 512], BF16, tag="ot")
                nc.vector.tensor_copy(ot[:], po[:])
                nc.sync.dma_start(
                    y_d[t * QT + qs * 128: t * QT + (qs + 1) * 128,
                        et * 512:(et + 1) * 512], ot[:])

            def pops(queue, qstate):
                if NO_INJECT:
                    return
                qstate[0] += qstate[1]
                while queue and qstate[0] >= 1.0:
                    qstate[0] -= 1.0
                    queue.pop(0)()

            def norm_unit(t, ep, accS):
                # aT = acc / rowsum; runs entirely off the attention
                # critical path (inputs already staged to SBUF).
                lrow = t1.tile([VW, 2, QT], F32R, tag="lrow")
                with nc.allow_low_precision(
                    reason="f32r tile holds full f32 bits"
                ):
                    nc.vector.reciprocal(lrow[DK:VW, :, :], accS[DK:VW, :, :])
                pbt = ps_pj.tile([128, 2, QT], F32, tag="pj")
                nc.tensor.matmul(pbt[0:DK, :, :], ones65[DK:VW, :],
                                 lrow[DK:VW, :, :], start=True, stop=True)
                rb = t1.tile([DK, 2, QT], F32, tag="rb")
                nc.vector.tensor_copy(rb[:], pbt[0:DK, :, :])
                aT = atp.tile([128, QT], BF16, tag=f"aT{ep}", name=f"aT{ep}_{t}")
                aT_tiles[(t, ep)] = aT
                nc.gpsimd.tensor_mul(aT[0:DK, :], accS[0:DK, 0, :], rb[:, 0, :])
                tmp = t1.tile([DK, QT], BF16, tag="tmp")
                nc.gpsimd.tensor_mul(tmp[:], accS[0:DK, 1, :], rb[:, 1, :])
                nc.sync.dma_start(aT[DK:128, :], tmp[:])

            def av_step(prev, kb):
                ep, acc, es = prev
                C = len(es)
                for h in range(2):
                    nc.tensor.matmul(
                        acc[h][:], vt[kb][:, 2 * ep + h, :], es[kb][:, h, :],
                        start=(kb == 0), stop=(kb == C - 1))

            def finish_phase(t, prev, queue):
                ep, acc, es = prev
                accS = t1.tile([VW, 2, QT], F32R, tag="accs")
                for h in range(2):
                    nc.vector.tensor_copy(accS[:, h, :], acc[h][:])
                queue.append(lambda: norm_unit(t, ep, accS))

            def attn_tile(t, queue, qstate):
                # scores/exp of head-pair ep run interleaved with the
                # AV accumulation of head-pair ep-1 (one phase behind),
                # so every AV's exp input is long since computed.
                C = 2 * (t + 1)
                qsl = slice(t * QT, (t + 1) * QT)
                prev = None
                for ep in range(NEP):
                    acc = [ps_ac.tile([VW, QT], F32, tag=f"ac{h}",
                                      name=f"acc{h}") for h in range(2)]
                    es = []
                    for kb in range(C):
                        e = t1.tile([128, 2, QT], BF16, tag="e", bufs=34)
                        psc = ps_sc.tile([128, 2, 512], F32, tag="sc", bufs=2)
                        for h in range(2):
                            pb = h * DK
                            nc.tensor.matmul(
                                psc[:, h, 0:QT],
                                krot[ep][pb:pb + DK, kb * KB:(kb + 1) * KB],
                                qrot[ep][pb:pb + DK, qsl],
                                start=True, stop=True,
                                tile_position=(pb, 0))
                        nc.scalar.activation(e[:], psc[:, :, 0:QT],
                                             mybir.ActivationFunctionType.Exp,
                                             scale=1.0 / math.sqrt(DK))
                        if kb >= C - 2:
                            em = t1.tile([128, 2, QT], BF16, tag="em", bufs=4)
                            nc.gpsimd.tensor_mul(em[:], e[:],
                                                 masks[:, kb - (C - 2), :, :])
                            e = em
                        es.append(e)
                        if prev is not None:
                            av_step(prev, kb)
                        pops(queue, qstate)
                    if prev is not None:
                        finish_phase(t, prev, queue)
                    prev = (ep, acc, es)
                for kb in range(C):
                    av_step(prev, kb)
                    pops(queue, qstate)
                finish_phase(t, prev, queue)

            # ---------------- main pipeline ----------------
            for fn in units_for_chunk(0):
                fn()

            for t in range(NT):
                queue = []
                if t + 3 <= NT - 1:
                    xs_dma(t + 3)
                if t + 1 < NT:
                    queue.extend(units_for_chunk(t + 1))
                if t >= 1:
                    for qs in range(2):
                        for et in range(2):
                            queue.append(
                                lambda t=t - 1, qs=qs, et=et: emit_po(t, qs, et))
                niter = (NEP + 1) * 2 * (t + 1)
                qstate = [0.0, (len(queue) + 4) / max(1, niter)]
                if NO_INJECT:
                    for fn in queue:
                        fn()
                    queue = []
                attn_tile(t, queue, qstate)
                while queue:
                    queue.pop(0)()
            for qs in range(2):
                for et in range(2):
                    emit_po(NT - 1, qs, et)

    nc.compile()
    nc.finalize()
    _cache["nc"] = nc
    return nc


def _rope_tables(pos):
    """cos/sin tables in [128, n] head-pair layout."""
    k = np.arange(DK // 2, dtype=np.float32)
    inv_freq = (THETA ** (-2.0 * k / DK)).astype(np.float32)
    ang = inv_freq[:, None] * pos.astype(np.float32)[None, :]  # [32, n]
    cos64 = np.repeat(np.cos(ang), 2, axis=0)
    sin64 = np.repeat(np.sin(ang), 2, axis=0)
    return (np.ascontiguousarray(
                np.concatenate([cos64, cos64], axis=0)).astype(np.float16),
            np.ascontiguousarray(
                np.concatenate([sin64, sin64], axis=0)).astype(np.float16))


def _masks():
    """[128, 2, 2, QT] bf16 multiplicative diagonal-block causal masks."""
    p = np.arange(KB)[:, None]
    f = np.arange(QT)[None, :]
    triA = (f >= p).astype(np.float32)
    triB = (f >= p + KB).astype(np.float32)
    m = np.stack([np.stack([triA, triA], 0), np.stack([triB, triB], 0)], 0)
    return np.ascontiguousarray(
        m.transpose(2, 0, 1, 3)).astype(ml_dtypes.bfloat16)


def _host_inputs(in_features, token_positions, Wq, Wk, Wv, Wo):
    X = np.asarray(in_features, dtype=np.float32)
    pos = np.asarray(token_positions)
    bf = ml_dtypes.bfloat16
    wqT = np.ascontiguousarray(np.asarray(Wq, np.float32).T).astype(bf)
    wkT = np.ascontiguousarray(np.asarray(Wk, np.float32).T).astype(bf)
    wvT = np.ascontiguousarray(np.asarray(Wv, np.float32).T).astype(bf)
    woT = np.ascontiguousarray(np.asarray(Wo, np.float32).T).astype(bf)
    cost, sint = _rope_tables(pos)

    permt = np.zeros((128, 128), np.float32)
    for i in range(64):
        permt[2 * i + 1, 2 * i] = -1.0
        permt[2 * i, 2 * i + 1] = 1.0

    mask = _masks()
    in_maps = []
    for core in range(8):
        b, j = core // 2, core % 2
        cs = slice(512 * j, 512 * (j + 1))
        in_maps.append({
            "xt": np.ascontiguousarray(X[b].T).astype(bf),
            "wkt": np.ascontiguousarray(wkT[:, cs]),
            "wvt": np.ascontiguousarray(wvT[:, cs]),
            "wqt": np.ascontiguousarray(wqT[:, cs]),
            "wot": np.ascontiguousarray(woT[cs, :]),
            "cost": cost, "sint": sint,
            "mask": mask, "permt": permt,
            "ones65": np.ones((VW, DK), np.float32),
        })
    return in_maps


def kernel(in_features, token_positions, Wq, Wk, Wv, Wo):
    nc = _build_program()
    in_maps = _host_inputs(in_features, token_positions, Wq, Wk, Wv, Wo)

    trace = bool(int(os.environ.get("KERNEL_TRACE", "0")))
    res = run_bass_kernel_spmd(nc, in_maps, core_ids=list(range(8)), trace=trace)
    kernel.last_result = res

    out = np.empty((B, S, D), np.float32)
    for b in range(B):
        out[b] = (res.results[2 * b]["y"].astype(np.float32)
                  + res.results[2 * b + 1]["y"].astype(np.float32))
    return out


# revision 19
# speedup vs baseline: 1.6822x; 1.0474x over previous
"""Causal multi-head self-attention with RoPE on 8 Trainium2 NeuronCores.

Sharding: batch (4) x head-half (2) -> 8 cores, no collectives.
Each core owns one batch element and 8 of the 16 heads (4 head-pairs of
128 partitions).  It computes K/V/Q projections for its heads only (no
redundant K/V work), full causal attention over all 8 query tiles, and a
row-sharded output projection producing a PARTIAL [S, D] output; the two
partials of a batch are summed on the host.

Pipeline: the program is emitted per 256-row sequence chunk c (== query
tile t).  Projection work for chunk t+1 is split into small sub-units and
injected into the attention inner loop of tile t, so the in-order PE
queue always has independent work between dependent attention steps.
The activation engine runs ONLY exp (both heads merged into one 512-wide
instruction reading a two-bank PSUM pair); rope/masking multiply on
Pool, PSUM-reading copies and normalization on DVE.

Layouts (transposed, feature-on-partition; no on-device transposes):
  K^T/Q^T = W^T.T @ X^T         bf16 inputs, fp32 PSUM accumulate
  RoPE    = cos*x + sin*(P@x)   (P = pair-rotation matrix) -> bf16
  S^T     = Krot^T-slice.T @ Qrot^T   (keys on partitions)
  exp     = ACT Exp(scale=1/8) over [128, 2heads, 256] -> bf16
  A^T,l   = [V|1].T @ exp       accumulated over key blocks in PSUM
  out     = A^T.T @ Wo^T        partial, host-reduced across head halves
"""

import os
import sys
import math

if "/opt/trn_rl_repo" not in sys.path:
    sys.path.append("/opt/trn_rl_repo")

import numpy as np
import ml_dtypes

import concourse.bass as bass
import concourse.tile as tile
from concourse import bacc, mybir
from concourse.bass_utils import run_bass_kernel_spmd

B = 4
S = 2048
D = 1024
H = 16
DK = 64
THETA = 10000.0

NEP = 4               # head pairs per core (8 heads)
QT = 256              # query tile width == chunk width
KB = 128              # key block
NT = S // QT          # 8 query tiles
VW = DK + 1           # V columns per head incl. trailing ones column

F32R = mybir.dt.float32r
F32 = mybir.dt.float32
F16 = mybir.dt.float16
BF16 = mybir.dt.bfloat16

_cache = {}
NO_INJECT = bool(int(os.environ.get("KERNEL_NO_INJECT", "0")))


def _build_program():
    if "nc" in _cache:
        return _cache["nc"]

    nc = bacc.Bacc("TRN2")

    xt_d = nc.dram_tensor("xt", [D, S], BF16, kind="ExternalInput")
    wkt_d = nc.dram_tensor("wkt", [D, NEP * 128], BF16, kind="ExternalInput")
    wvt_d = nc.dram_tensor("wvt", [D, NEP * 128], BF16, kind="ExternalInput")
    wqt_d = nc.dram_tensor("wqt", [D, NEP * 128], BF16, kind="ExternalInput")
    wot_d = nc.dram_tensor("wot", [NEP * 128, D], BF16, kind="ExternalInput")
    cost_d = nc.dram_tensor("cost", [128, S], F16, kind="ExternalInput")
    sint_d = nc.dram_tensor("sint", [128, S], F16, kind="ExternalInput")
    mask_d = nc.dram_tensor("mask", [128, 2, 2, QT], BF16, kind="ExternalInput")
    permt_d = nc.dram_tensor("permt", [128, 128], F32R, kind="ExternalInput")
    ones_d = nc.dram_tensor("ones65", [VW, DK], F32R, kind="ExternalInput")
    y_d = nc.dram_tensor("y", [S, D], BF16, kind="ExternalOutput")

    xt_t = xt_d.rearrange("(n p) s -> p n s", p=128)      # [128, 8, S]
    wkt_t = wkt_d.rearrange("(n p) e -> p n e", p=128)    # [128, 8, 512]
    wqt_t = wqt_d.rearrange("(n p) e -> p n e", p=128)
    wvt_t = wvt_d.rearrange("(n p) e -> p n e", p=128)
    wot_t = wot_d.rearrange("(n p) e -> p n e", p=128)    # [128, 4, 1024]

    with tile.TileContext(nc) as tc:
        with (
            tc.tile_pool(name="const", bufs=1) as cpool,
            tc.tile_pool(name="wpool", bufs=1) as wpool,
            tc.tile_pool(name="kv", bufs=1) as kv,
            tc.tile_pool(name="xsp", bufs=3) as xsp,
            tc.tile_pool(name="t1", bufs=2) as t1,
            tc.tile_pool(name="atp", bufs=2) as atp,
            tc.tile_pool(name="ps_sc", bufs=2, space="PSUM") as ps_sc,
            tc.tile_pool(name="ps_pj", bufs=2, space="PSUM") as ps_pj,
            tc.tile_pool(name="ps_ac", bufs=1, space="PSUM") as ps_ac,
        ):
            # ---------------- persistent tiles ----------------
            permt = cpool.tile([128, 128], F32R)
            ones65 = cpool.tile([VW, DK], F32R)
            cost = cpool.tile([128, S], F16)
            sint = cpool.tile([128, S], F16)
            masks = cpool.tile([128, 2, 2, QT], BF16)

            wk = [wpool.tile([128, 8, 128], BF16, tag=f"wk{e}", name=f"wk{e}")
                  for e in range(NEP)]
            wq = [wpool.tile([128, 8, 128], BF16, tag=f"wq{e}", name=f"wq{e}")
                  for e in range(NEP)]
            wv = [wpool.tile([128, NEP * 128], BF16, tag=f"wv{d}", name=f"wv{d}")
                  for d in range(8)]
            wo = [wpool.tile([128, D], BF16, tag=f"wo{d}", name=f"wo{d}")
                  for d in range(NEP)]

            krot = [kv.tile([128, S], BF16, tag=f"krot{e}", name=f"krot{e}")
                    for e in range(NEP)]
            qrot = [kv.tile([128, S], BF16, tag=f"qrot{e}", name=f"qrot{e}")
                    for e in range(NEP)]
            vt = [kv.tile([128, 8, VW], BF16, tag=f"vt{kb}", name=f"vt{kb}")
                  for kb in range(S // KB)]

            # ---------------- prologue DMAs ----------------
            xs_tiles = {}

            def xs_dma(c):
                xs = xsp.tile([128, 8, QT], BF16, tag="xs", name=f"xs{c}")
                nc.sync.dma_start(xs[:], xt_t[:, :, c * QT:(c + 1) * QT])
                xs_tiles[c] = xs

            nc.sync.dma_start(wk[0][:], wkt_t[:, :, 0:128])
            xs_dma(0)
            nc.sync.dma_start(cost[:], cost_d[:])
            nc.sync.dma_start(sint[:], sint_d[:])
            nc.sync.dma_start(permt[:], permt_d[:])
            nc.sync.dma_start(wk[1][:], wkt_t[:, :, 128:256])
            for d in range(4):
                nc.sync.dma_start(wv[d][:], wvt_t[:, d, :])
            nc.sync.dma_start(wk[2][:], wkt_t[:, :, 256:384])
            for d in range(4, 8):
                nc.sync.dma_start(wv[d][:], wvt_t[:, d, :])
            nc.sync.dma_start(wk[3][:], wkt_t[:, :, 384:512])
            for e in range(NEP):
                nc.sync.dma_start(wq[e][:], wqt_t[:, :, e * 128:(e + 1) * 128])
            nc.sync.dma_start(masks[:], mask_d[:])
            nc.sync.dma_start(ones65[:], ones_d[:])
            xs_dma(1)
            xs_dma(2)
            for d in range(NEP):
                nc.sync.dma_start(wo[d][:], wot_t[:, d, :])
            for kb in range(S // KB):
                nc.vector.memset(vt[kb][:, :, DK], 1.0)

            # ---------------- unit emitters ----------------
            # K/Q projection is split into two sub-units so the PSUM->SBUF
            # copy latency is hidden behind other injected PE work.
            raws = {}

            def proj_a(c, ep, wtile, raw_tag):
                pk = ps_pj.tile([128, 512], F32, tag="pj")
                for d in range(8):
                    nc.tensor.matmul(pk[:, 0:QT], wtile[:, d, :],
                                     xs_tiles[c][:, d, :],
                                     start=(d == 0), stop=(d == 7))
                raw = t1.tile([128, QT], F32R, tag=raw_tag)
                nc.vector.tensor_copy(raw[:], pk[:, 0:QT])
                raws[(c, ep, raw_tag)] = raw

            def proj_b(c, ep, out, raw_tag):
                csl = slice(c * QT, (c + 1) * QT)
                raw = raws.pop((c, ep, raw_tag))
                pp = ps_pj.tile([128, 512], F32, tag="pj")
                nc.tensor.matmul(pp[:, 0:QT], permt[:], raw[:],
                                 start=True, stop=True)
                t_c = t1.tile([128, QT], BF16, tag="tc")
                nc.gpsimd.tensor_mul(t_c[:], raw[:], cost[:, csl])
                t_s = t1.tile([128, QT], BF16, tag="ts")
                nc.vector.tensor_mul(t_s[:], pp[:, 0:QT], sint[:, csl])
                nc.gpsimd.tensor_add(out[:, csl], t_c[:], t_s[:])

            def emit_V(c, half):
                kb = 2 * c + half
                off = half * KB
                pv = ps_pj.tile([128, 512], F32, tag="pj")
                for d in range(8):
                    nc.tensor.matmul(pv[:], xs_tiles[c][:, d, off:off + KB],
                                     wv[d][:], start=(d == 0), stop=(d == 7))
                nc.vector.tensor_copy(
                    vt[kb][:, :, 0:DK],
                    pv[:].rearrange("p (h w) -> p h w", w=DK))

            def units_for_chunk(c):
                us = []
                for ep in range(NEP):
                    us.append(lambda c=c, ep=ep: proj_a(c, ep, wk[ep], "kraw"))
                    us.append(lambda c=c, ep=ep: proj_b(c, ep, krot[ep], "kraw"))
                us.insert(2, lambda c=c: emit_V(c, 0))
                us.insert(6, lambda c=c: emit_V(c, 1))
                for ep in range(NEP):
                    us.append(lambda c=c, ep=ep: proj_a(c, ep, wq[ep], "qraw"))
                    us.append(lambda c=c, ep=ep: proj_b(c, ep, qrot[ep], "qraw"))
                return us

            aT_tiles = {}

            def emit_po(t, qs, et):
                po = ps_pj.tile([128, 512], F32, tag="pj")
                for d in range(NEP):
                    nc.tensor.matmul(
                        po[:], aT_tiles[(t, d)][:, qs * 128:(qs + 1) * 128],
                        wo[d][:, et * 512:(et + 1) * 512],
                        start=(d == 0), stop=(d == NEP - 1))
                ot = t1.tile([128, 512], BF16, tag="ot")
                nc.vector.tensor_copy(ot[:], po[:])
                nc.sync.dma_start(
                    y_d[t * QT + qs * 128: t * QT + (qs + 1) * 128,
                        et * 512:(et + 1) * 512], ot[:])

            def pops(queue, qstate):
                if NO_INJECT:
                    return
                qstate[0] += qstate[1]
                while queue and qstate[0] >= 1.0:
                    qstate[0] -= 1.0
                    queue.pop(0)()

            def norm_unit(t, ep, accS):
                # aT = acc / rowsum; runs entirely off the attention
                # critical path (inputs already staged to SBUF).
                lrow = t1.tile([VW, 2, QT], F32R, tag="lrow")
                with nc.allow_low_precision(
                    reason="f32r tile holds full f32 bits"
                ):
                    nc.vector.reciprocal(lrow[DK:VW, :, :], accS[DK:VW, :, :])
                pbt = ps_pj.tile([128, 2, QT], F32, tag="pj")
                nc.tensor.matmul(pbt[0:DK, :, :], ones65[DK:VW, :],
                                 lrow[DK:VW, :, :], start=True, stop=True)
                rb = t1.tile([DK, 2, QT], F32, tag="rb")
                nc.vector.tensor_copy(rb[:], pbt[0:DK, :, :])
                aT = atp.tile([128, QT], BF16, tag=f"aT{ep}", name=f"aT{ep}_{t}")
                aT_tiles[(t, ep)] = aT
                nc.gpsimd.tensor_mul(aT[0:DK, :], accS[0:DK, 0, :], rb[:, 0, :])
                tmp = t1.tile([DK, QT], BF16, tag="tmp")
                nc.gpsimd.tensor_mul(tmp[:], accS[0:DK, 1, :], rb[:, 1, :])
                nc.sync.dma_start(aT[DK:128, :], tmp[:])

            def av_step(prev, kb):
                ep, acc, es = prev
                C = len(es)
                for h in range(2):
                    nc.tensor.matmul(
                        acc[h][:], vt[kb][:, 2 * ep + h, :], es[kb][:, h, :],
                        start=(kb == 0), stop=(kb == C - 1))

            def finish_phase(t, prev, queue):
                ep, acc, es = prev
                accS = t1.tile([VW, 2, QT], F32R, tag="accs")
                for h in range(2):
                    nc.vector.tensor_copy(accS[:, h, :], acc[h][:])
                queue.append(lambda: norm_unit(t, ep, accS))

            def attn_tile(t, queue, qstate):
                # scores/exp of head-pair ep run interleaved with the
                # AV accumulation of head-pair ep-1 (one phase behind),
                # so every AV's exp input is long since computed.
                C = 2 * (t + 1)
                qsl = slice(t * QT, (t + 1) * QT)
                prev = None
                for ep in range(NEP):
                    acc = [ps_ac.tile([VW, QT], F32, tag=f"ac{h}",
                                      name=f"acc{h}") for h in range(2)]
                    es = []
                    for kb in range(C):
                        e = t1.tile([128, 2, QT], BF16, tag="e", bufs=34)
                        psc = ps_sc.tile([128, 2, 512], F32, tag="sc", bufs=2)
                        for h in range(2):
                            pb = h * DK
                            nc.tensor.matmul(
                                psc[:, h, 0:QT],
                                krot[ep][pb:pb + DK, kb * KB:(kb + 1) * KB],
                                qrot[ep][pb:pb + DK, qsl],
                                start=True, stop=True,
                                tile_position=(pb, 0))
                        nc.scalar.activation(e[:], psc[:, :, 0:QT],
                                             mybir.ActivationFunctionType.Exp,
                                             scale=1.0 / math.sqrt(DK))
                        if kb >= C - 2:
                            em = t1.tile([128, 2, QT], BF16, tag="em", bufs=4)
                            nc.vector.tensor_mul(em[:], e[:],
                                                 masks[:, kb - (C - 2), :, :])
                            e = em
                        es.append(e)
                        if prev is not None:
                            av_step(prev, kb)
                        pops(queue, qstate)
                    if prev is not None:
                        finish_phase(t, prev, queue)
                    prev = (ep, acc, es)
                for kb in range(C):
                    av_step(prev, kb)
                    pops(queue, qstate)
                finish_phase(t, prev, queue)

            # ---------------- main pipeline ----------------
            for fn in units_for_chunk(0):
                fn()

            for t in range(NT):
                queue = []
                if t + 3 <= NT - 1:
                    xs_dma(t + 3)
                if t + 1 < NT:
                    queue.extend(units_for_chunk(t + 1))
                if t >= 1:
                    for qs in range(2):
                        for et in range(2):
                            queue.append(
                                lambda t=t - 1, qs=qs, et=et: emit_po(t, qs, et))
                niter = (NEP + 1) * 2 * (t + 1)
                qstate = [0.0, (len(queue) + 4) / max(1, niter)]
                if NO_INJECT:
                    for fn in queue:
                        fn()
                    queue = []
                attn_tile(t, queue, qstate)
                while queue:
                    queue.pop(0)()
            for qs in range(2):
                for et in range(2):
                    emit_po(NT - 1, qs, et)

    nc.compile()
    nc.finalize()
    _cache["nc"] = nc
    return nc


def _rope_tables(pos):
    """cos/sin tables in [128, n] head-pair layout."""
    k = np.arange(DK // 2, dtype=np.float32)
    inv_freq = (THETA ** (-2.0 * k / DK)).astype(np.float32)
    ang = inv_freq[:, None] * pos.astype(np.float32)[None, :]  # [32, n]
    cos64 = np.repeat(np.cos(ang), 2, axis=0)
    sin64 = np.repeat(np.sin(ang), 2, axis=0)
    return (np.ascontiguousarray(
                np.concatenate([cos64, cos64], axis=0)).astype(np.float16),
            np.ascontiguousarray(
                np.concatenate([sin64, sin64], axis=0)).astype(np.float16))


def _masks():
    """[128, 2, 2, QT] bf16 multiplicative diagonal-block causal masks."""
    p = np.arange(KB)[:, None]
    f = np.arange(QT)[None, :]
    triA = (f >= p).astype(np.float32)
    triB = (f >= p + KB).astype(np.float32)
    m = np.stack([np.stack([triA, triA], 0), np.stack([triB, triB], 0)], 0)
    return np.ascontiguousarray(
        m.transpose(2, 0, 1, 3)).astype(ml_dtypes.bfloat16)


def _host_inputs(in_features, token_positions, Wq, Wk, Wv, Wo):
    X = np.asarray(in_features, dtype=np.float32)
    pos = np.asarray(token_positions)
    bf = ml_dtypes.bfloat16
    wqT = np.ascontiguousarray(np.asarray(Wq, np.float32).T).astype(bf)
    wkT = np.ascontiguousarray(np.asarray(Wk, np.float32).T).astype(bf)
    wvT = np.ascontiguousarray(np.asarray(Wv, np.float32).T).astype(bf)
    woT = np.ascontiguousarray(np.asarray(Wo, np.float32).T).astype(bf)
    cost, sint = _rope_tables(pos)

    permt = np.zeros((128, 128), np.float32)
    for i in range(64):
        permt[2 * i + 1, 2 * i] = -1.0
        permt[2 * i, 2 * i + 1] = 1.0

    mask = _masks()
    in_maps = []
    for core in range(8):
        b, j = core // 2, core % 2
        cs = slice(512 * j, 512 * (j + 1))
        in_maps.append({
            "xt": np.ascontiguousarray(X[b].T).astype(bf),
            "wkt": np.ascontiguousarray(wkT[:, cs]),
            "wvt": np.ascontiguousarray(wvT[:, cs]),
            "wqt": np.ascontiguousarray(wqT[:, cs]),
            "wot": np.ascontiguousarray(woT[cs, :]),
            "cost": cost, "sint": sint,
            "mask": mask, "permt": permt,
            "ones65": np.ones((VW, DK), np.float32),
        })
    return in_maps


def kernel(in_features, token_positions, Wq, Wk, Wv, Wo):
    nc = _build_program()
    in_maps = _host_inputs(in_features, token_positions, Wq, Wk, Wv, Wo)

    trace = bool(int(os.environ.get("KERNEL_TRACE", "0")))
    res = run_bass_kernel_spmd(nc, in_maps, core_ids=list(range(8)), trace=trace)
    kernel.last_result = res

    out = np.empty((B, S, D), np.float32)
    for b in range(B):
        out[b] = (res.results[2 * b]["y"].astype(np.float32)
                  + res.results[2 * b + 1]["y"].astype(np.float32))
    return out
